# revision 37
# baseline (speedup 1.0000x reference)
"""Trainium2 Bass kernel for causal MHA (B=2, T=2048, D=1024, H=16, KH=64).

Sharding: 8 cores = 2 (batch) x 4 (head groups of 4 heads).
Each core computes q/k/v projections for its 4 heads, causal attention,
and a partial output projection against its 256-row slice of Wout.
Host sums the 4 partials per batch (the all-reduce step, done at unshard).

Tuned for the PE HAM clock gate (the PE only reaches 2.4 GHz after
~3.4us of gapless execution, so every engine queue must stay saturated):
bf16 matmul pipeline, consolidated early DMAs, score PSUM tiles
pair-fused across the two heads of a pair so one Scalar EXP covers both,
V projection interleaved into the attention stream, softmax
normalization off the PE critical path (S broadcast by a PE rank-1
matmul, fast approximate reciprocal on DVE), and an output projection
with full 128-partition stationaries (head pairs stacked along
partitions, odd heads assembled via SBUF-to-SBUF DMA), with bf16
partial outputs summed on the host.
"""
import sys

sys.path.insert(0, "/opt/trn_rl_repo")

from contextlib import ExitStack

import numpy as np

import concourse.bacc as bacc
import concourse.mybir as mybir
import concourse.tile as tile

B, T, C = 2, 2048, 1024
H, KH = 16, 64
G = 4                 # head groups
HPG = H // G          # heads per group = 4
DG = HPG * KH         # 256 per-core head dims
NCORES = 8

F32 = mybir.dt.float32
F32R = mybir.dt.float32r
BF16 = mybir.dt.bfloat16
EXP = mybir.ActivationFunctionType.Exp
COPY = mybir.ActivationFunctionType.Copy

_cached_nc = None


def build_nc():
    nc = bacc.Bacc()
    xt = nc.dram_tensor("xt", [C, T], BF16, kind="ExternalInput")        # x[b].T
    wq = nc.dram_tensor("wq", [C, DG], BF16, kind="ExternalInput")       # Wq slice .T
    wk = nc.dram_tensor("wk", [C, DG], BF16, kind="ExternalInput")
    wv = nc.dram_tensor("wv", [C, DG], BF16, kind="ExternalInput")
    wo = nc.dram_tensor("wo", [DG, C], BF16, kind="ExternalInput")       # Wout[:, slice].T
    keep = nc.dram_tensor("keep", [128, T], BF16, kind="ExternalInput")  # diag keep blocks (k, q)
    y = nc.dram_tensor("y", [T, C], BF16, kind="ExternalOutput")         # partial output

    NT = T // 512     # 4 moving t tiles
    NK = C // 128     # 8 contraction chunks
    NTT = T // 128    # 16 t tiles of 128

    with ExitStack() as ctx:
        ctx.enter_context(nc.allow_low_precision(reason="bf16 matmul pipeline"))
        tc = ctx.enter_context(tile.TileContext(nc))
        persist = ctx.enter_context(tc.tile_pool(name="persist", bufs=1))
        psum = ctx.enter_context(tc.tile_pool(name="psum", bufs=2, space="PSUM"))

        # ---- persistent tiles ----
        qT = [persist.tile([128, T], BF16, tag=f"qT{i}", name=f"qT{i}") for i in range(2)]
        kT = [persist.tile([128, T], BF16, tag=f"kT{i}", name=f"kT{i}") for i in range(2)]
        vsbA = persist.tile([128, NTT, HPG, KH + 1], BF16, tag="vsbA")
        # aT2[hp]: normalized attention output for head pair hp, heads
        # stacked along partitions (h even -> 0:64, h odd -> 64:128).
        aT2 = [persist.tile([128, T], BF16, tag=f"aT2{i}", name=f"aT2{i}")
               for i in range(2)]
        a_hi = [persist.tile([64, T], BF16, tag=f"a_hi{i}", name=f"a_hi{i}")
                for i in range(2)]
        wo_sb = persist.tile([128, 2, C], BF16, tag="wo_sb")
        keep_sb = persist.tile([128, T], BF16, tag="keep")
        ones_sb = persist.tile([65, 64], F32R, tag="ones")
        ones_f32 = persist.tile([65, 64], F32, tag="ones_f32")

        # ---- phase 1 input tiles (kept until end of phase 1) ----
        ph1 = ctx.enter_context(tc.tile_pool(name="ph1", bufs=1))
        xTa = ph1.tile([128, NK, T], BF16, tag="xTa", name="xTa")
        wq_sb = ph1.tile([128, NK, DG], BF16, tag="wq_sb")
        wk_sb = ph1.tile([128, NK, DG], BF16, tag="wk_sb")
        wv_sb = ph1.tile([128, NK, DG], BF16, tag="wv_sb")

        # Consolidated DMAs, issue spread across four sequencer queues so the
        # ~600ns-per-descriptor issue cost is paid in parallel and phase 1
        # can start as early as possible.
        engs = [nc.sync, nc.scalar, nc.gpsimd]
        nc.sync.dma_start(out=wq_sb, in_=wq.rearrange("(k p) d -> p k d", k=NK))
        for k in range(NK):
            engs[(k + 1) % 3].dma_start(out=xTa[:, k, :],
                                        in_=xt[k * 128:(k + 1) * 128, :])
        nc.scalar.dma_start(out=wk_sb, in_=wk.rearrange("(k p) d -> p k d", k=NK))
        nc.gpsimd.dma_start(out=wv_sb, in_=wv.rearrange("(k p) d -> p k d", k=NK))
        nc.gpsimd.dma_start(out=keep_sb, in_=keep[:, :])
        nc.sync.dma_start(out=wo_sb, in_=wo.rearrange("(h p) c -> p h c", h=2))
        nc.vector.memset(ones_f32, 1.0)
        nc.vector.tensor_copy(out=ones_sb, in_=ones_f32)

        # ================= Phase 1: projections =================
        # qT/kT: (dk 128-pair, t) = sum_c w[c, dk].T . xT[c, t]
        # Two 512-wide t tiles share one 2-bank PSUM tile -> one drain each.
        for dst, w_sb in ((qT, wq_sb), (kT, wk_sb)):
            for m in range(2):          # head pair -> partition block
                for np_ in range(NT // 2):
                    ps = psum.tile([128, 1024], F32, tag="scA", name="ps")
                    for k in range(NK):
                        for half in range(2):
                            n = 2 * np_ + half
                            nc.tensor.matmul(
                                ps[:, half * 512:(half + 1) * 512],
                                w_sb[:, k, m * 128:(m + 1) * 128],
                                xTa[:, k, n * 512:(n + 1) * 512],
                                start=(k == 0), stop=(k == NK - 1),
                            )
                    nc.vector.tensor_copy(
                        out=dst[m][:, np_ * 1024:(np_ + 1) * 1024], in_=ps)
        # V: (t 128, dv 256) = sum_c xT[c, t].T . wv[c, dv]; four 128-t tiles
        # per PSUM tile, one packed drain (+ ones cols for the S trick).
        # Emitted lazily inside the attention stream (group gtt right before
        # q block gtt's first unit) — V matmuls add no Scalar work, so they
        # fill PE slack while EXPs stream.
        def emit_vgroup(gtt):
            ps = psum.tile([128, 1024], F32, tag="scA", name="ps")
            for j in range(4):
                tt = 4 * gtt + j
                for k in range(NK):
                    nc.tensor.matmul(
                        ps[:, j * 256:(j + 1) * 256],
                        xTa[:, k, tt * 128:(tt + 1) * 128],
                        wv_sb[:, k, :],
                        start=(k == 0), stop=(k == NK - 1),
                    )
            nc.vector.tensor_copy(
                out=vsbA[:, 4 * gtt:4 * gtt + 4, :, 0:KH],
                in_=ps[:].rearrange("p (q h d) -> p q h d", q=4, h=HPG),
            )
            nc.vector.memset(vsbA[:, 4 * gtt:4 * gtt + 4, :, KH:KH + 1], 1.0)

        # ================= Phase 2: attention =================
        # Unit = (qj, hp): a 512-wide q block for a head pair. Score matmuls
        # write both heads into one 2-bank PSUM tile; a single EXP drains it
        # to a pair-fused pt tile. PV of unit i is emitted after the scores
        # of unit i+1; normalization trails one more unit so the PE queue
        # always has ready matmuls (keeps the HAM clock gate open).
        with tc.tile_pool(name="pts", bufs=1) as ptp, \
             tc.tile_pool(name="srowp", bufs=2) as srp:

            units = []
            for qj in range(NT):
                for hp in range(2):
                    units.append({"qj": qj, "hp": hp, "kmax": 4 * qj + 4})

            def emit_sc_block(u):
                qj, hp = u["qj"], u["hp"]
                u["pts"] = []
                for kt in range(u["kmax"]):
                    off = 128 * (kt - 4 * qj) if kt >= 4 * qj else 0
                    sc = psum.tile([128, 1024], F32, tag="scA", name="sc")
                    for par in range(2):
                        nc.tensor.matmul(
                            sc[:, par * 512 + off:(par + 1) * 512],
                            kT[hp][64 * par:64 * par + 64, kt * 128:(kt + 1) * 128],
                            qT[hp][64 * par:64 * par + 64,
                                   qj * 512 + off:(qj + 1) * 512],
                            start=True, stop=True,
                        )
                    pt = ptp.tile([128, 1024], BF16, tag="pt", bufs=33, name="pt")
                    nc.scalar.activation(
                        out=pt[:].rearrange("p (g c) -> p g c", g=2)[:, :, off:512],
                        in_=sc[:].rearrange("p (g c) -> p g c", g=2)[:, :, off:512],
                        func=EXP, scale=0.125)
                    if kt >= 4 * qj:
                        for par in range(2):
                            nc.vector.tensor_mul(
                                pt[:, par * 512 + off:par * 512 + off + 128],
                                pt[:, par * 512 + off:par * 512 + off + 128],
                                keep_sb[:, kt * 128:(kt + 1) * 128],
                            )
                    u["pts"].append((pt, off))

            def emit_pv_block(u):
                qj, hp, kmax = u["qj"], u["hp"], u["kmax"]
                u["acc"] = [psum.tile([65, 512], F32, tag=f"acc{par}",
                                      name=f"acc{par}") for par in range(2)]
                for par in range(2):
                    h = 2 * hp + par
                    for kt in range(kmax):
                        pt, off = u["pts"][kt]
                        nc.tensor.matmul(
                            u["acc"][par][:, off:512],
                            vsbA[:, kt, h, :],
                            pt[:, par * 512 + off:(par + 1) * 512],
                            start=(kt == 0), stop=(kt == kmax - 1),
                        )
                u["pts"] = None

            def emit_norm_a(u):
                # Copy the S rows (partition 64) of each acc into SBUF so the
                # broadcast matmul can use them as moving operands.
                srows = []
                for par in range(2):
                    srow = srp.tile([65, 512], F32R, tag=f"srow{par}",
                                    name="srow")
                    nc.vector.tensor_copy(out=srow[64:65, :],
                                          in_=u["acc"][par][64:65, :])
                    srows.append(srow)
                u["srows"] = srows

            def emit_norm_b(u):
                # Broadcast S across 64 partitions via PE, one fast reciprocal
                # on DVE, then fused (acc * 1/S) -> bf16 into the stacked aT2
                # (even head direct; odd head staged then DMA'd to 64:128).
                qj, hp = u["qj"], u["hp"]
                rbcp = psum.tile([64, 1024], F32, tag="scA", name="rbcp")
                for par in range(2):
                    nc.tensor.matmul(
                        rbcp[:, par * 512:(par + 1) * 512],
                        ones_sb[64:65, :],
                        u["srows"][par][64:65, :],
                        start=True, stop=True)
                rec = srp.tile([64, 1024], F32, tag="rec", name="rec")
                nc.vector.reciprocal_approx_fast(out=rec, in_=rbcp)
                nc.vector.tensor_mul(
                    aT2[hp][0:64, qj * 512:(qj + 1) * 512],
                    u["acc"][0][0:64, :],
                    rec[:, 0:512],
                )
                nc.vector.tensor_mul(
                    a_hi[hp][:, qj * 512:(qj + 1) * 512],
                    u["acc"][1][0:64, :],
                    rec[:, 512:1024],
                )
                nc.sync.dma_start(
                    out=aT2[hp][64:128, qj * 512:(qj + 1) * 512],
                    in_=a_hi[hp][:, qj * 512:(qj + 1) * 512],
                )

            pv_pend = None
            for u in units:
                if u["hp"] == 0:
                    emit_vgroup(u["qj"])
                emit_sc_block(u)
                if pv_pend is not None:
                    emit_pv_block(pv_pend)
                    emit_norm_a(pv_pend)
                    emit_norm_b(pv_pend)
                pv_pend = u
            emit_pv_block(pv_pend)
            emit_norm_a(pv_pend)
            emit_norm_b(pv_pend)

        # ================= Phase 3: output projection =================
        # y[t, c] = sum_{hp} aT2[hp][:, t].T @ wo2[hp]; full 128-partition
        # stationaries, two 512-wide c halves per 2-bank PSUM tile, drains
        # alternating between DVE and GpSimd so neither paces the PE.
        with tc.tile_pool(name="ph3", bufs=4) as ph3:
            for tt in range(NTT):
                yt = ph3.tile([128, C], BF16, tag="ysb", name="yt")
                if tt % 2 == 0:
                    yp = psum.tile([128, 1024], F32, tag="scA", name="yp")
                    yph = [yp[:, 0:512], yp[:, 512:1024]]
                else:
                    yph = [psum.tile([128, 512], F32, tag=f"acc{no}", name="yp")
                           for no in range(2)]
                for no in range(2):
                    for hp in range(2):
                        nc.tensor.matmul(
                            yph[no],
                            aT2[hp][:, tt * 128:(tt + 1) * 128],
                            wo_sb[:, hp, no * 512:(no + 1) * 512],
                            start=(hp == 0), stop=(hp == 1),
                        )
                nc.vector.tensor_copy(out=yt[:, 0:512], in_=yph[0])
                nc.scalar.activation(out=yt[:, 512:1024], in_=yph[1],
                                     func=COPY)
                nc.sync.dma_start(out=y[tt * 128:(tt + 1) * 128, :], in_=yt)

    _split_excess_waits(nc)
    nc.compile()
    return nc


def _split_excess_waits(nc):
    """Walrus caps most instructions at 1 sync wait. Peel excess waits off
    matmuls (and anything else over the cap) onto PE-engine wait-nops
    inserted immediately before the instruction."""
    for bb in nc.main_func.blocks:
        new_insts = []
        for inst in bb.instructions:
            si = inst.sync_info
            if (si is not None and si.on_wait and len(si.on_wait) > 1
                    and isinstance(inst, mybir.InstMatmult)):
                excess = list(si.on_wait[:-1])
                keep = [si.on_wait[-1]]
                for w in excess:
                    nop = mybir.InstNoOp(
                        name=nc.get_next_instruction_name(), ins=[], outs=[],
                        bass_nofuse=True)
                    nop.engine = inst.engine
                    nop.sync_info = mybir.SyncInfo(on_wait=[w], on_update=[])
                    nc.register_instruction(nop)
                    new_insts.append(nop)
                si.on_wait = keep
            new_insts.append(inst)
        bb.instructions[:] = new_insts


def _host_prep(x, Wq, Wkv, Wout, mask):
    import ml_dtypes
    BF = ml_dtypes.bfloat16

    x = np.asarray(x, dtype=np.float32)
    Wq = np.asarray(Wq, dtype=np.float32)
    Wkv = np.asarray(Wkv, dtype=np.float32)
    Wout = np.asarray(Wout, dtype=np.float32)
    mask = np.asarray(mask)

    xT = [np.ascontiguousarray(x[b].T).astype(BF) for b in range(B)]
    keep = np.empty((128, T), dtype=np.float32)
    for i in range(T // 128):
        blk = mask[128 * i:128 * (i + 1), 128 * i:128 * (i + 1)]
        keep[:, 128 * i:128 * (i + 1)] = (~blk).T.astype(np.float32)
    keep = keep.astype(BF)

    in_maps = []
    for core in range(NCORES):
        b, g = core // G, core % G
        sl = slice(DG * g, DG * (g + 1))
        in_maps.append({
            "xt": xT[b],
            "wq": np.ascontiguousarray(Wq[sl, :].T).astype(BF),
            "wk": np.ascontiguousarray(Wkv[sl, :].T).astype(BF),
            "wv": np.ascontiguousarray(Wkv[C + DG * g:C + DG * (g + 1), :].T).astype(BF),
            "wo": np.ascontiguousarray(Wout[:, sl].T).astype(BF),
            "keep": keep,
        })
    return in_maps


def _install_ntff_hook():
    import types
    import antenv
    if getattr(antenv, "axon_hooks", None) is not None:
        return
    ah = types.ModuleType("antenv.axon_hooks")
    ah._hook = None
    ah.set_axon_ntff_profile_hook = lambda h: setattr(ah, "_hook", h)
    ah.get_axon_ntff_profile_hook = lambda: ah._hook
    sys.modules["antenv.axon_hooks"] = ah
    antenv.axon_hooks = ah
    if "/root/.axon_site" not in sys.path:
        sys.path.insert(0, "/root/.axon_site")
    from trn_agent_boot.trn_boot import _ntff_profile_via_ctypes
    ah.set_axon_ntff_profile_hook(_ntff_profile_via_ctypes("/opt/axon/libaxon_pjrt.so"))


def _run(inputs, trace=False):
    global _cached_nc
    from concourse.bass_utils import run_bass_kernel_spmd
    if trace:
        _install_ntff_hook()
    if _cached_nc is None:
        _cached_nc = build_nc()
    in_maps = _host_prep(**inputs)
    res = run_bass_kernel_spmd(_cached_nc, in_maps, list(range(NCORES)), trace=trace)
    parts = [np.asarray(res.results[c]["y"], dtype=np.float32)
             for c in range(NCORES)]
    out = np.stack([
        parts[0] + parts[1] + parts[2] + parts[3],
        parts[4] + parts[5] + parts[6] + parts[7],
    ]).astype(np.float32)
    return out, res


def kernel(x, Wq, Wkv, Wout, mask):
    out, _ = _run(dict(x=x, Wq=Wq, Wkv=Wkv, Wout=Wout, mask=mask))
    return out


# revision 39
# speedup vs baseline: 1.1423x; 1.1423x over previous
"""Trainium2 Bass kernel for causal MHA (B=2, T=2048, D=1024, H=16, KH=64).

Sharding: 8 cores = 2 (batch) x 4 (head groups of 4 heads).
Each core computes q/k/v projections for its 4 heads, causal attention,
and a partial output projection against its 256-row slice of Wout.
Host sums the 4 partials per batch (the all-reduce step, done at unshard).

Tuned for the PE HAM clock gate (the PE only reaches 2.4 GHz after
~3.4us of gapless execution, so every engine queue must stay saturated):
bf16 matmul pipeline, consolidated early DMAs, score PSUM tiles
pair-fused across the two heads of a pair so one Scalar EXP covers both,
V projection interleaved into the attention stream, softmax
normalization off the PE critical path (S broadcast by a PE rank-1
matmul, fast approximate reciprocal on DVE), and an output projection
with full 128-partition stationaries (head pairs stacked along
partitions, odd heads assembled via SBUF-to-SBUF DMA), with bf16
partial outputs summed on the host.
"""
import sys

sys.path.insert(0, "/opt/trn_rl_repo")

from contextlib import ExitStack

import numpy as np

import concourse.bacc as bacc
import concourse.mybir as mybir
import concourse.tile as tile

B, T, C = 2, 2048, 1024
H, KH = 16, 64
G = 4                 # head groups
HPG = H // G          # heads per group = 4
DG = HPG * KH         # 256 per-core head dims
NCORES = 8

F32 = mybir.dt.float32
F32R = mybir.dt.float32r
BF16 = mybir.dt.bfloat16
EXP = mybir.ActivationFunctionType.Exp
COPY = mybir.ActivationFunctionType.Copy

_cached_nc = None


def build_nc():
    nc = bacc.Bacc()
    xt = nc.dram_tensor("xt", [C, T], BF16, kind="ExternalInput")        # x[b].T
    wq = nc.dram_tensor("wq", [C, DG], BF16, kind="ExternalInput")       # Wq slice .T
    wk = nc.dram_tensor("wk", [C, DG], BF16, kind="ExternalInput")
    wv = nc.dram_tensor("wv", [C, DG], BF16, kind="ExternalInput")
    wo = nc.dram_tensor("wo", [DG, C], BF16, kind="ExternalInput")       # Wout[:, slice].T
    keep = nc.dram_tensor("keep", [128, T], BF16, kind="ExternalInput")  # diag keep blocks (k, q)
    y = nc.dram_tensor("y", [T, C], BF16, kind="ExternalOutput")         # partial output

    NT = T // 512     # 4 moving t tiles
    NK = C // 128     # 8 contraction chunks
    NTT = T // 128    # 16 t tiles of 128

    with ExitStack() as ctx:
        ctx.enter_context(nc.allow_low_precision(reason="bf16 matmul pipeline"))
        tc = ctx.enter_context(tile.TileContext(nc))
        persist = ctx.enter_context(tc.tile_pool(name="persist", bufs=1))
        psum = ctx.enter_context(tc.tile_pool(name="psum", bufs=2, space="PSUM"))

        # ---- persistent tiles ----
        qT = [persist.tile([128, T], BF16, tag=f"qT{i}", name=f"qT{i}") for i in range(2)]
        kT = [persist.tile([128, T], BF16, tag=f"kT{i}", name=f"kT{i}") for i in range(2)]
        vsbA = persist.tile([128, NTT, HPG, KH + 1], BF16, tag="vsbA")
        # aT2[hp]: normalized attention output for head pair hp, heads
        # stacked along partitions (h even -> 0:64, h odd -> 64:128).
        aT2 = [persist.tile([128, T], BF16, tag=f"aT2{i}", name=f"aT2{i}")
               for i in range(2)]
        a_hi = [persist.tile([64, T], BF16, tag=f"a_hi{i}", name=f"a_hi{i}")
                for i in range(2)]
        wo_sb = persist.tile([128, 2, C], BF16, tag="wo_sb")
        keep_sb = persist.tile([128, T], BF16, tag="keep")
        ones_sb = persist.tile([65, 64], F32R, tag="ones")
        ones_f32 = persist.tile([65, 64], F32, tag="ones_f32")

        # ---- phase 1 input tiles (kept until end of phase 1) ----
        ph1 = ctx.enter_context(tc.tile_pool(name="ph1", bufs=1))
        xTa = ph1.tile([128, NK, T], BF16, tag="xTa", name="xTa")
        wq_sb = ph1.tile([128, NK, DG], BF16, tag="wq_sb")
        wk_sb = ph1.tile([128, NK, DG], BF16, tag="wk_sb")
        wv_sb = ph1.tile([128, NK, DG], BF16, tag="wv_sb")

        # Consolidated DMAs, issue spread across four sequencer queues so the
        # ~600ns-per-descriptor issue cost is paid in parallel and phase 1
        # can start as early as possible.
        nc.sync.dma_start(out=wq_sb, in_=wq.rearrange("(k p) d -> p k d", k=NK))
        for k in range(NK):
            nc.sync.dma_start(out=xTa[:, k, :],
                              in_=xt[k * 128:(k + 1) * 128, :])
        nc.sync.dma_start(out=wk_sb, in_=wk.rearrange("(k p) d -> p k d", k=NK))
        nc.sync.dma_start(out=wv_sb, in_=wv.rearrange("(k p) d -> p k d", k=NK))
        nc.sync.dma_start(out=keep_sb, in_=keep[:, :])
        nc.sync.dma_start(out=wo_sb, in_=wo.rearrange("(h p) c -> p h c", h=2))
        nc.vector.memset(ones_f32, 1.0)
        nc.vector.tensor_copy(out=ones_sb, in_=ones_f32)

        # ================= Phase 1: projections =================
        # qT/kT: (dk 128-pair, t) = sum_c w[c, dk].T . xT[c, t]
        # Two 512-wide t tiles share one 2-bank PSUM tile -> one drain each.
        for dst, w_sb in ((qT, wq_sb), (kT, wk_sb)):
            for m in range(2):          # head pair -> partition block
                for np_ in range(NT // 2):
                    ps = psum.tile([128, 1024], F32, tag="scA", name="ps")
                    for k in range(NK):
                        for half in range(2):
                            n = 2 * np_ + half
                            nc.tensor.matmul(
                                ps[:, half * 512:(half + 1) * 512],
                                w_sb[:, k, m * 128:(m + 1) * 128],
                                xTa[:, k, n * 512:(n + 1) * 512],
                                start=(k == 0), stop=(k == NK - 1),
                            )
                    nc.vector.tensor_copy(
                        out=dst[m][:, np_ * 1024:(np_ + 1) * 1024], in_=ps)
        # V: (t 128, dv 256) = sum_c xT[c, t].T . wv[c, dv]; four 128-t tiles
        # per PSUM tile, one packed drain (+ ones cols for the S trick).
        # Emitted lazily inside the attention stream (group gtt right before
        # q block gtt's first unit) — V matmuls add no Scalar work, so they
        # fill PE slack while EXPs stream.
        def emit_vgroup(gtt):
            ps = psum.tile([128, 1024], F32, tag="scA", name="ps")
            for j in range(4):
                tt = 4 * gtt + j
                for k in range(NK):
                    nc.tensor.matmul(
                        ps[:, j * 256:(j + 1) * 256],
                        xTa[:, k, tt * 128:(tt + 1) * 128],
                        wv_sb[:, k, :],
                        start=(k == 0), stop=(k == NK - 1),
                    )
            nc.vector.tensor_copy(
                out=vsbA[:, 4 * gtt:4 * gtt + 4, :, 0:KH],
                in_=ps[:].rearrange("p (q h d) -> p q h d", q=4, h=HPG),
            )
            nc.vector.memset(vsbA[:, 4 * gtt:4 * gtt + 4, :, KH:KH + 1], 1.0)

        # ================= Phase 2: attention =================
        # Unit = (qj, hp): a 512-wide q block for a head pair. Score matmuls
        # write both heads into one 2-bank PSUM tile; a single EXP drains it
        # to a pair-fused pt tile. PV of unit i is emitted after the scores
        # of unit i+1; normalization trails one more unit so the PE queue
        # always has ready matmuls (keeps the HAM clock gate open).
        with tc.tile_pool(name="pts", bufs=1) as ptp, \
             tc.tile_pool(name="srowp", bufs=2) as srp:

            units = []
            for qj in range(NT):
                for hp in range(2):
                    units.append({"qj": qj, "hp": hp, "kmax": 4 * qj + 4})

            def emit_sc_block(u):
                qj, hp = u["qj"], u["hp"]
                u["pts"] = []
                for kt in range(u["kmax"]):
                    off = 128 * (kt - 4 * qj) if kt >= 4 * qj else 0
                    sc = psum.tile([128, 1024], F32, tag="scA", name="sc")
                    for par in range(2):
                        nc.tensor.matmul(
                            sc[:, par * 512 + off:(par + 1) * 512],
                            kT[hp][64 * par:64 * par + 64, kt * 128:(kt + 1) * 128],
                            qT[hp][64 * par:64 * par + 64,
                                   qj * 512 + off:(qj + 1) * 512],
                            start=True, stop=True,
                        )
                    pt = ptp.tile([128, 1024], BF16, tag="pt", bufs=33, name="pt")
                    nc.scalar.activation(
                        out=pt[:].rearrange("p (g c) -> p g c", g=2)[:, :, off:512],
                        in_=sc[:].rearrange("p (g c) -> p g c", g=2)[:, :, off:512],
                        func=EXP, scale=0.125)
                    if kt >= 4 * qj:
                        for par in range(2):
                            nc.vector.tensor_mul(
                                pt[:, par * 512 + off:par * 512 + off + 128],
                                pt[:, par * 512 + off:par * 512 + off + 128],
                                keep_sb[:, kt * 128:(kt + 1) * 128],
                            )
                    u["pts"].append((pt, off))

            def emit_pv_block(u):
                qj, hp, kmax = u["qj"], u["hp"], u["kmax"]
                u["acc"] = [psum.tile([65, 512], F32, tag=f"acc{par}",
                                      name=f"acc{par}") for par in range(2)]
                # par-inner so consecutive PV matmuls alternate PSUM banks
                # (back-to-back accumulation into one bank adds ~110ns each).
                for kt in range(kmax):
                    pt, off = u["pts"][kt]
                    for par in range(2):
                        h = 2 * hp + par
                        nc.tensor.matmul(
                            u["acc"][par][:, off:512],
                            vsbA[:, kt, h, :],
                            pt[:, par * 512 + off:(par + 1) * 512],
                            start=(kt == 0), stop=(kt == kmax - 1),
                        )
                u["pts"] = None

            def emit_norm_a(u):
                # Copy the S rows (partition 64) of each acc into SBUF so the
                # broadcast matmul can use them as moving operands.
                srows = []
                for par in range(2):
                    srow = srp.tile([65, 512], F32R, tag=f"srow{par}",
                                    name="srow")
                    nc.vector.tensor_copy(out=srow[64:65, :],
                                          in_=u["acc"][par][64:65, :])
                    srows.append(srow)
                u["srows"] = srows

            def emit_norm_b(u):
                # Broadcast S across 64 partitions via PE, one fast reciprocal
                # on DVE, then fused (acc * 1/S) -> bf16 into the stacked aT2
                # (even head direct; odd head staged then DMA'd to 64:128).
                qj, hp = u["qj"], u["hp"]
                rbcp = psum.tile([64, 1024], F32, tag="scA", name="rbcp")
                for par in range(2):
                    nc.tensor.matmul(
                        rbcp[:, par * 512:(par + 1) * 512],
                        ones_sb[64:65, :],
                        u["srows"][par][64:65, :],
                        start=True, stop=True)
                rec = srp.tile([64, 1024], F32, tag="rec", name="rec")
                nc.vector.reciprocal_approx_fast(out=rec, in_=rbcp)
                nc.vector.tensor_mul(
                    aT2[hp][0:64, qj * 512:(qj + 1) * 512],
                    u["acc"][0][0:64, :],
                    rec[:, 0:512],
                )
                nc.vector.tensor_mul(
                    a_hi[hp][:, qj * 512:(qj + 1) * 512],
                    u["acc"][1][0:64, :],
                    rec[:, 512:1024],
                )
                nc.sync.dma_start(
                    out=aT2[hp][64:128, qj * 512:(qj + 1) * 512],
                    in_=a_hi[hp][:, qj * 512:(qj + 1) * 512],
                )

            pv_pend = None
            for u in units:
                if u["hp"] == 0:
                    emit_vgroup(u["qj"])
                emit_sc_block(u)
                if pv_pend is not None:
                    emit_pv_block(pv_pend)
                    emit_norm_a(pv_pend)
                    emit_norm_b(pv_pend)
                pv_pend = u
            emit_pv_block(pv_pend)
            emit_norm_a(pv_pend)
            emit_norm_b(pv_pend)

        # ================= Phase 3: output projection =================
        # y[t, c] = sum_{hp} aT2[hp][:, t].T @ wo2[hp]; full 128-partition
        # stationaries, two 512-wide c halves per 2-bank PSUM tile, drains
        # alternating between DVE and GpSimd so neither paces the PE.
        with tc.tile_pool(name="ph3", bufs=4) as ph3:
            for tt in range(NTT):
                yt = ph3.tile([128, C], BF16, tag="ysb", name="yt")
                if tt % 2 == 0:
                    yp = psum.tile([128, 1024], F32, tag="scA", name="yp")
                    yph = [yp[:, 0:512], yp[:, 512:1024]]
                else:
                    yph = [psum.tile([128, 512], F32, tag=f"acc{no}", name="yp")
                           for no in range(2)]
                for no in range(2):
                    for hp in range(2):
                        nc.tensor.matmul(
                            yph[no],
                            aT2[hp][:, tt * 128:(tt + 1) * 128],
                            wo_sb[:, hp, no * 512:(no + 1) * 512],
                            start=(hp == 0), stop=(hp == 1),
                        )
                nc.vector.tensor_copy(out=yt[:, 0:512], in_=yph[0])
                nc.scalar.activation(out=yt[:, 512:1024], in_=yph[1],
                                     func=COPY)
                nc.sync.dma_start(out=y[tt * 128:(tt + 1) * 128, :], in_=yt)

    _split_excess_waits(nc)
    nc.compile()
    return nc


def _split_excess_waits(nc):
    """Walrus caps most instructions at 1 sync wait. Peel excess waits off
    matmuls (and anything else over the cap) onto PE-engine wait-nops
    inserted immediately before the instruction."""
    for bb in nc.main_func.blocks:
        new_insts = []
        for inst in bb.instructions:
            si = inst.sync_info
            if (si is not None and si.on_wait and len(si.on_wait) > 1
                    and isinstance(inst, mybir.InstMatmult)):
                excess = list(si.on_wait[:-1])
                keep = [si.on_wait[-1]]
                for w in excess:
                    nop = mybir.InstNoOp(
                        name=nc.get_next_instruction_name(), ins=[], outs=[],
                        bass_nofuse=True)
                    nop.engine = inst.engine
                    nop.sync_info = mybir.SyncInfo(on_wait=[w], on_update=[])
                    nc.register_instruction(nop)
                    new_insts.append(nop)
                si.on_wait = keep
            new_insts.append(inst)
        bb.instructions[:] = new_insts


def _host_prep(x, Wq, Wkv, Wout, mask):
    import ml_dtypes
    BF = ml_dtypes.bfloat16

    x = np.asarray(x, dtype=np.float32)
    Wq = np.asarray(Wq, dtype=np.float32)
    Wkv = np.asarray(Wkv, dtype=np.float32)
    Wout = np.asarray(Wout, dtype=np.float32)
    mask = np.asarray(mask)

    xT = [np.ascontiguousarray(x[b].T).astype(BF) for b in range(B)]
    keep = np.empty((128, T), dtype=np.float32)
    for i in range(T // 128):
        blk = mask[128 * i:128 * (i + 1), 128 * i:128 * (i + 1)]
        keep[:, 128 * i:128 * (i + 1)] = (~blk).T.astype(np.float32)
    keep = keep.astype(BF)

    in_maps = []
    for core in range(NCORES):
        b, g = core // G, core % G
        sl = slice(DG * g, DG * (g + 1))
        in_maps.append({
            "xt": xT[b],
            "wq": np.ascontiguousarray(Wq[sl, :].T).astype(BF),
            "wk": np.ascontiguousarray(Wkv[sl, :].T).astype(BF),
            "wv": np.ascontiguousarray(Wkv[C + DG * g:C + DG * (g + 1), :].T).astype(BF),
            "wo": np.ascontiguousarray(Wout[:, sl].T).astype(BF),
            "keep": keep,
        })
    return in_maps


def _install_ntff_hook():
    import types
    import antenv
    if getattr(antenv, "axon_hooks", None) is not None:
        return
    ah = types.ModuleType("antenv.axon_hooks")
    ah._hook = None
    ah.set_axon_ntff_profile_hook = lambda h: setattr(ah, "_hook", h)
    ah.get_axon_ntff_profile_hook = lambda: ah._hook
    sys.modules["antenv.axon_hooks"] = ah
    antenv.axon_hooks = ah
    if "/root/.axon_site" not in sys.path:
        sys.path.insert(0, "/root/.axon_site")
    from trn_agent_boot.trn_boot import _ntff_profile_via_ctypes
    ah.set_axon_ntff_profile_hook(_ntff_profile_via_ctypes("/opt/axon/libaxon_pjrt.so"))


def _run(inputs, trace=False):
    global _cached_nc
    from concourse.bass_utils import run_bass_kernel_spmd
    if trace:
        _install_ntff_hook()
    if _cached_nc is None:
        _cached_nc = build_nc()
    in_maps = _host_prep(**inputs)
    res = run_bass_kernel_spmd(_cached_nc, in_maps, list(range(NCORES)), trace=trace)
    parts = [np.asarray(res.results[c]["y"], dtype=np.float32)
             for c in range(NCORES)]
    out = np.stack([
        parts[0] + parts[1] + parts[2] + parts[3],
        parts[4] + parts[5] + parts[6] + parts[7],
    ]).astype(np.float32)
    return out, res


def kernel(x, Wq, Wkv, Wout, mask):
    out, _ = _run(dict(x=x, Wq=Wq, Wkv=Wkv, Wout=Wout, mask=mask))
    return out


# revision 40
# speedup vs baseline: 1.1702x; 1.0244x over previous
"""Trainium2 Bass kernel for causal MHA (B=2, T=2048, D=1024, H=16, KH=64).

Sharding: 8 cores = 2 (batch) x 4 (head groups of 4 heads).
Each core computes q/k/v projections for its 4 heads, causal attention,
and a partial output projection against its 256-row slice of Wout.
Host sums the 4 partials per batch (the all-reduce step, done at unshard).

Tuned for the PE HAM clock gate (the PE only reaches 2.4 GHz after
~3.4us of gapless execution, so every engine queue must stay saturated):
bf16 matmul pipeline, consolidated early DMAs, score PSUM tiles
pair-fused across the two heads of a pair so one Scalar EXP covers both,
V projection interleaved into the attention stream, softmax
normalization off the PE critical path (S broadcast by a PE rank-1
matmul, fast approximate reciprocal on DVE), and an output projection
with full 128-partition stationaries (head pairs stacked along
partitions, odd heads assembled via SBUF-to-SBUF DMA), with bf16
partial outputs summed on the host.
"""
import sys

sys.path.insert(0, "/opt/trn_rl_repo")

from contextlib import ExitStack

import numpy as np

import concourse.bacc as bacc
import concourse.mybir as mybir
import concourse.tile as tile

B, T, C = 2, 2048, 1024
H, KH = 16, 64
G = 4                 # head groups
HPG = H // G          # heads per group = 4
DG = HPG * KH         # 256 per-core head dims
NCORES = 8

F32 = mybir.dt.float32
F32R = mybir.dt.float32r
BF16 = mybir.dt.bfloat16
EXP = mybir.ActivationFunctionType.Exp
COPY = mybir.ActivationFunctionType.Copy

_cached_nc = None


def build_nc():
    nc = bacc.Bacc()
    xt = nc.dram_tensor("xt", [C, T], BF16, kind="ExternalInput")        # x[b].T
    wq = nc.dram_tensor("wq", [C, DG], BF16, kind="ExternalInput")       # Wq slice .T
    wk = nc.dram_tensor("wk", [C, DG], BF16, kind="ExternalInput")
    wv = nc.dram_tensor("wv", [C, DG], BF16, kind="ExternalInput")
    wo = nc.dram_tensor("wo", [DG, C], BF16, kind="ExternalInput")       # Wout[:, slice].T
    keep = nc.dram_tensor("keep", [128, T], BF16, kind="ExternalInput")  # diag keep blocks (k, q)
    y = nc.dram_tensor("y", [T, C], BF16, kind="ExternalOutput")         # partial output

    NT = T // 512     # 4 moving t tiles
    NK = C // 128     # 8 contraction chunks
    NTT = T // 128    # 16 t tiles of 128

    with ExitStack() as ctx:
        ctx.enter_context(nc.allow_low_precision(reason="bf16 matmul pipeline"))
        tc = ctx.enter_context(tile.TileContext(nc))
        persist = ctx.enter_context(tc.tile_pool(name="persist", bufs=1))
        psum = ctx.enter_context(tc.tile_pool(name="psum", bufs=2, space="PSUM"))

        # ---- persistent tiles ----
        qT = [persist.tile([128, T], BF16, tag=f"qT{i}", name=f"qT{i}") for i in range(2)]
        kT = [persist.tile([128, T], BF16, tag=f"kT{i}", name=f"kT{i}") for i in range(2)]
        vsbA = persist.tile([128, NTT, HPG, 128], BF16, tag="vsbA")
        # aT2[hp]: normalized attention output for head pair hp, heads
        # stacked along partitions (h even -> 0:64, h odd -> 64:128).
        aT2 = [persist.tile([128, T], BF16, tag=f"aT2{i}", name=f"aT2{i}")
               for i in range(2)]
        a_hi = [persist.tile([64, T], BF16, tag=f"a_hi{i}", name=f"a_hi{i}")
                for i in range(2)]
        wo_sb = persist.tile([128, 2, C], BF16, tag="wo_sb")
        keep_sb = persist.tile([128, T], BF16, tag="keep")
        ones_sb = persist.tile([65, 64], F32R, tag="ones")
        ones_f32 = persist.tile([65, 64], F32, tag="ones_f32")

        # ---- phase 1 input tiles (kept until end of phase 1) ----
        ph1 = ctx.enter_context(tc.tile_pool(name="ph1", bufs=1))
        xTa = ph1.tile([128, NK, T], BF16, tag="xTa", name="xTa")
        wq_sb = ph1.tile([128, NK, DG], BF16, tag="wq_sb")
        wk_sb = ph1.tile([128, NK, DG], BF16, tag="wk_sb")
        wv_sb = ph1.tile([128, NK, DG], BF16, tag="wv_sb")

        # Consolidated DMAs, issue spread across four sequencer queues so the
        # ~600ns-per-descriptor issue cost is paid in parallel and phase 1
        # can start as early as possible.
        nc.sync.dma_start(out=wq_sb, in_=wq.rearrange("(k p) d -> p k d", k=NK))
        for k in range(NK):
            nc.sync.dma_start(out=xTa[:, k, :],
                              in_=xt[k * 128:(k + 1) * 128, :])
        nc.sync.dma_start(out=wk_sb, in_=wk.rearrange("(k p) d -> p k d", k=NK))
        nc.sync.dma_start(out=wv_sb, in_=wv.rearrange("(k p) d -> p k d", k=NK))
        nc.sync.dma_start(out=keep_sb, in_=keep[:, :])
        nc.sync.dma_start(out=wo_sb, in_=wo.rearrange("(h p) c -> p h c", h=2))
        nc.vector.memset(ones_f32, 1.0)
        nc.vector.tensor_copy(out=ones_sb, in_=ones_f32)

        # ================= Phase 1: projections =================
        # qT/kT: (dk 128-pair, t) = sum_c w[c, dk].T . xT[c, t]
        # Two 512-wide t tiles share one 2-bank PSUM tile -> one drain each.
        for dst, w_sb in ((qT, wq_sb), (kT, wk_sb)):
            for m in range(2):          # head pair -> partition block
                for np_ in range(NT // 2):
                    ps = psum.tile([128, 1024], F32, tag="scA", name="ps")
                    for k in range(NK):
                        for half in range(2):
                            n = 2 * np_ + half
                            nc.tensor.matmul(
                                ps[:, half * 512:(half + 1) * 512],
                                w_sb[:, k, m * 128:(m + 1) * 128],
                                xTa[:, k, n * 512:(n + 1) * 512],
                                start=(k == 0), stop=(k == NK - 1),
                            )
                    nc.vector.tensor_copy(
                        out=dst[m][:, np_ * 1024:(np_ + 1) * 1024], in_=ps)
        # V: (t 128, dv 256) = sum_c xT[c, t].T . wv[c, dv]; four 128-t tiles
        # per PSUM tile, one packed drain (+ ones cols for the S trick).
        # Emitted lazily inside the attention stream (group gtt right before
        # q block gtt's first unit) — V matmuls add no Scalar work, so they
        # fill PE slack while EXPs stream.
        def emit_vgroup(gtt):
            ps = psum.tile([128, 1024], F32, tag="scA", name="ps")
            for j in range(4):
                tt = 4 * gtt + j
                for k in range(NK):
                    nc.tensor.matmul(
                        ps[:, j * 256:(j + 1) * 256],
                        xTa[:, k, tt * 128:(tt + 1) * 128],
                        wv_sb[:, k, :],
                        start=(k == 0), stop=(k == NK - 1),
                    )
            nc.vector.tensor_copy(
                out=vsbA[:, 4 * gtt:4 * gtt + 4, :, 0:KH],
                in_=ps[:].rearrange("p (q h d) -> p q h d", q=4, h=HPG),
            )
            nc.vector.memset(vsbA[:, 4 * gtt:4 * gtt + 4, :, KH:KH + 1], 1.0)

        # ================= Phase 2: attention =================
        # Unit = (qj, hp): a 512-wide q block for a head pair. Score matmuls
        # write both heads into one 2-bank PSUM tile; a single EXP drains it
        # to a pair-fused pt tile. PV of unit i is emitted after the scores
        # of unit i+1; normalization trails one more unit so the PE queue
        # always has ready matmuls (keeps the HAM clock gate open).
        with tc.tile_pool(name="pts", bufs=1) as ptp, \
             tc.tile_pool(name="srowp", bufs=2) as srp:

            units = []
            for qj in range(NT):
                for hp in range(2):
                    units.append({"qj": qj, "hp": hp, "kmax": 4 * qj + 4})

            def emit_sc_block(u):
                qj, hp = u["qj"], u["hp"]
                u["pts"] = []
                for kt in range(u["kmax"]):
                    off = 128 * (kt - 4 * qj) if kt >= 4 * qj else 0
                    sc = psum.tile([128, 1024], F32, tag="scA", name="sc")
                    for par in range(2):
                        nc.tensor.matmul(
                            sc[:, par * 512 + off:(par + 1) * 512],
                            kT[hp][64 * par:64 * par + 64, kt * 128:(kt + 1) * 128],
                            qT[hp][64 * par:64 * par + 64,
                                   qj * 512 + off:(qj + 1) * 512],
                            start=True, stop=True,
                        )
                    pt = ptp.tile([128, 1024], BF16, tag="pt", bufs=33, name="pt")
                    nc.scalar.activation(
                        out=pt[:].rearrange("p (g c) -> p g c", g=2)[:, :, off:512],
                        in_=sc[:].rearrange("p (g c) -> p g c", g=2)[:, :, off:512],
                        func=EXP, scale=0.125)
                    if kt >= 4 * qj:
                        for par in range(2):
                            nc.vector.tensor_mul(
                                pt[:, par * 512 + off:par * 512 + off + 128],
                                pt[:, par * 512 + off:par * 512 + off + 128],
                                keep_sb[:, kt * 128:(kt + 1) * 128],
                            )
                    u["pts"].append((pt, off))

            def emit_pv_block(u):
                qj, hp, kmax = u["qj"], u["hp"], u["kmax"]
                u["acc"] = [psum.tile([65, 512], F32, tag=f"acc{par}",
                                      name=f"acc{par}") for par in range(2)]
                for par in range(2):
                    h = 2 * hp + par
                    for kt in range(kmax):
                        pt, off = u["pts"][kt]
                        nc.tensor.matmul(
                            u["acc"][par][:, off:512],
                            vsbA[:, kt, h, 0:KH + 1],
                            pt[:, par * 512 + off:(par + 1) * 512],
                            start=(kt == 0), stop=(kt == kmax - 1),
                        )
                u["pts"] = None

            def emit_norm_a(u):
                # Copy the S rows (partition 64) of each acc into SBUF so the
                # broadcast matmul can use them as moving operands.
                srows = []
                for par in range(2):
                    srow = srp.tile([65, 512], F32R, tag=f"srow{par}",
                                    name="srow")
                    nc.vector.tensor_copy(out=srow[64:65, :],
                                          in_=u["acc"][par][64:65, :])
                    srows.append(srow)
                u["srows"] = srows

            def emit_norm_b(u):
                # Broadcast S across 64 partitions via PE, one fast reciprocal
                # on DVE, then fused (acc * 1/S) -> bf16 into the stacked aT2
                # (even head direct; odd head staged then DMA'd to 64:128).
                qj, hp = u["qj"], u["hp"]
                rbcp = psum.tile([64, 1024], F32, tag="scA", name="rbcp")
                for par in range(2):
                    nc.tensor.matmul(
                        rbcp[:, par * 512:(par + 1) * 512],
                        ones_sb[64:65, :],
                        u["srows"][par][64:65, :],
                        start=True, stop=True)
                rec = srp.tile([64, 1024], F32, tag="rec", name="rec")
                nc.vector.reciprocal_approx_fast(out=rec, in_=rbcp)
                nc.vector.tensor_mul(
                    aT2[hp][0:64, qj * 512:(qj + 1) * 512],
                    u["acc"][0][0:64, :],
                    rec[:, 0:512],
                )
                nc.vector.tensor_mul(
                    a_hi[hp][:, qj * 512:(qj + 1) * 512],
                    u["acc"][1][0:64, :],
                    rec[:, 512:1024],
                )
                nc.sync.dma_start(
                    out=aT2[hp][64:128, qj * 512:(qj + 1) * 512],
                    in_=a_hi[hp][:, qj * 512:(qj + 1) * 512],
                )

            pv_pend = None
            for u in units:
                if u["hp"] == 0:
                    emit_vgroup(u["qj"])
                emit_sc_block(u)
                if pv_pend is not None:
                    emit_pv_block(pv_pend)
                    emit_norm_a(pv_pend)
                    emit_norm_b(pv_pend)
                pv_pend = u
            emit_pv_block(pv_pend)
            emit_norm_a(pv_pend)
            emit_norm_b(pv_pend)

        # ================= Phase 3: output projection =================
        # y[t, c] = sum_{hp} aT2[hp][:, t].T @ wo2[hp]; full 128-partition
        # stationaries, two 512-wide c halves per 2-bank PSUM tile, drains
        # alternating between DVE and GpSimd so neither paces the PE.
        with tc.tile_pool(name="ph3", bufs=4) as ph3:
            for tt in range(NTT):
                yt = ph3.tile([128, C], BF16, tag="ysb", name="yt")
                if tt % 2 == 0:
                    yp = psum.tile([128, 1024], F32, tag="scA", name="yp")
                    yph = [yp[:, 0:512], yp[:, 512:1024]]
                else:
                    yph = [psum.tile([128, 512], F32, tag=f"acc{no}", name="yp")
                           for no in range(2)]
                for no in range(2):
                    for hp in range(2):
                        nc.tensor.matmul(
                            yph[no],
                            aT2[hp][:, tt * 128:(tt + 1) * 128],
                            wo_sb[:, hp, no * 512:(no + 1) * 512],
                            start=(hp == 0), stop=(hp == 1),
                        )
                nc.vector.tensor_copy(out=yt[:, 0:512], in_=yph[0])
                nc.scalar.activation(out=yt[:, 512:1024], in_=yph[1],
                                     func=COPY)
                nc.sync.dma_start(out=y[tt * 128:(tt + 1) * 128, :], in_=yt)

    _split_excess_waits(nc)
    nc.compile()
    return nc


def _split_excess_waits(nc):
    """Walrus caps most instructions at 1 sync wait. Peel excess waits off
    matmuls (and anything else over the cap) onto PE-engine wait-nops
    inserted immediately before the instruction."""
    for bb in nc.main_func.blocks:
        new_insts = []
        for inst in bb.instructions:
            si = inst.sync_info
            if (si is not None and si.on_wait and len(si.on_wait) > 1
                    and isinstance(inst, mybir.InstMatmult)):
                excess = list(si.on_wait[:-1])
                keep = [si.on_wait[-1]]
                for w in excess:
                    nop = mybir.InstNoOp(
                        name=nc.get_next_instruction_name(), ins=[], outs=[],
                        bass_nofuse=True)
                    nop.engine = inst.engine
                    nop.sync_info = mybir.SyncInfo(on_wait=[w], on_update=[])
                    nc.register_instruction(nop)
                    new_insts.append(nop)
                si.on_wait = keep
            new_insts.append(inst)
        bb.instructions[:] = new_insts


def _host_prep(x, Wq, Wkv, Wout, mask):
    import ml_dtypes
    BF = ml_dtypes.bfloat16

    x = np.asarray(x, dtype=np.float32)
    Wq = np.asarray(Wq, dtype=np.float32)
    Wkv = np.asarray(Wkv, dtype=np.float32)
    Wout = np.asarray(Wout, dtype=np.float32)
    mask = np.asarray(mask)

    xT = [np.ascontiguousarray(x[b].T).astype(BF) for b in range(B)]
    keep = np.empty((128, T), dtype=np.float32)
    for i in range(T // 128):
        blk = mask[128 * i:128 * (i + 1), 128 * i:128 * (i + 1)]
        keep[:, 128 * i:128 * (i + 1)] = (~blk).T.astype(np.float32)
    keep = keep.astype(BF)

    in_maps = []
    for core in range(NCORES):
        b, g = core // G, core % G
        sl = slice(DG * g, DG * (g + 1))
        in_maps.append({
            "xt": xT[b],
            "wq": np.ascontiguousarray(Wq[sl, :].T).astype(BF),
            "wk": np.ascontiguousarray(Wkv[sl, :].T).astype(BF),
            "wv": np.ascontiguousarray(Wkv[C + DG * g:C + DG * (g + 1), :].T).astype(BF),
            "wo": np.ascontiguousarray(Wout[:, sl].T).astype(BF),
            "keep": keep,
        })
    return in_maps


def _install_ntff_hook():
    import types
    import antenv
    if getattr(antenv, "axon_hooks", None) is not None:
        return
    ah = types.ModuleType("antenv.axon_hooks")
    ah._hook = None
    ah.set_axon_ntff_profile_hook = lambda h: setattr(ah, "_hook", h)
    ah.get_axon_ntff_profile_hook = lambda: ah._hook
    sys.modules["antenv.axon_hooks"] = ah
    antenv.axon_hooks = ah
    if "/root/.axon_site" not in sys.path:
        sys.path.insert(0, "/root/.axon_site")
    from trn_agent_boot.trn_boot import _ntff_profile_via_ctypes
    ah.set_axon_ntff_profile_hook(_ntff_profile_via_ctypes("/opt/axon/libaxon_pjrt.so"))


def _run(inputs, trace=False):
    global _cached_nc
    from concourse.bass_utils import run_bass_kernel_spmd
    if trace:
        _install_ntff_hook()
    if _cached_nc is None:
        _cached_nc = build_nc()
    in_maps = _host_prep(**inputs)
    res = run_bass_kernel_spmd(_cached_nc, in_maps, list(range(NCORES)), trace=trace)
    parts = [np.asarray(res.results[c]["y"], dtype=np.float32)
             for c in range(NCORES)]
    out = np.stack([
        parts[0] + parts[1] + parts[2] + parts[3],
        parts[4] + parts[5] + parts[6] + parts[7],
    ]).astype(np.float32)
    return out, res


def kernel(x, Wq, Wkv, Wout, mask):
    out, _ = _run(dict(x=x, Wq=Wq, Wkv=Wkv, Wout=Wout, mask=mask))
    return out
